# revision 46
# baseline (speedup 1.0000x reference)
"""Trainium2 Bass kernel for dense_cnn problem.

Math (per batch element n, C=128 channels, H=W=56, G=8):
  t1 = conv_h(x, w1)          5-tap conv over H with full channel mixing
  t3 = dwconv_h(t1, w3)       3-tap depthwise conv over H
  t4[g] = sum_{c,k} x[c, h, w+2k-2] * w4[c,k,g]   (3 width taps, dil 2)
  out[c] = t3[c] * t4[c % 8]

Device strategy (data-parallel, 4 batch elems per core across 8 cores).
The PE matmul stream is the critical path (1 column/cycle @2.4GHz), so
the structure minimizes PE columns subject to the other engines keeping
up (their measured capacities are much lower than the docs' rooflines,
and they run several us behind their emission points near stream end):

  - Elements 0..2 (hybrid): rows 0..46 use plain 5-tap t1 matmuls; the
    3-tap depthwise conv runs on DVE as two ~24-row units (three
    per-partition-scaled tensor_scalar reads + two adds -- these hit
    the DVE fast bf16 mode; scalar_tensor_tensor does not) with the
    t3*t4 combine on the Pool engine.  These units complete mid-stream
    while the PE works on later elements.  Rows 47..55 use the folded
    7-tap t3 = (w3*w1) conv directly (one PSUM bank).  A third
    depthwise unit instead of the fold overloads DVE/Pool and stalls
    the PE (measured) -- this split balances them.
  - Element 3 (processed last) is FULLY folded: 7-tap t3 per 8-row
    chunk, ScalarE copies t4 to bf16, DVE multiplies.  Any depthwise
    unit emitted near stream end lands squarely in the tail; the fold
    keeps the tail to a copy+mul+DMA of the final rows, split 4+4 so
    only a 4-row transfer trails the last matmul.
  - t4: w4 broadcast to 128 output channels on the host -> 3 width-tap
    matmuls per chunk; the final combine is elementwise.
  - Matmuls in bf16; accumulation fp32 in PSUM.  x unpadded in SBUF;
    boundary taps are clipped matmuls.  Output DMA'd bf16 (half the
    bytes) and upconverted on the host.
  - x loads for elements 1-3 are issued mid-stream: issuing them up
    front steals HBM bandwidth from the pieces the PE needs first, and
    the resulting matmul stalls also reset the PE_HAM activity window,
    delaying the 1.2 -> 2.4 GHz clock boost.
  - 8 dummy 448-col matmuls (sized to end right as the first x piece
    and weights land, ~10.5us in) trip the PE_HAM clock gate while the
    first DMAs are still streaming in.
"""

import sys

sys.path.insert(0, "/opt/trn_rl_repo")

import ml_dtypes
import numpy as np

import concourse.bacc as bacc
import concourse.bass as bass
import concourse.mybir as mybir
import concourse.tile as tile
from concourse import bass_utils

N, C, H, W, G = 32, 128, 56, 56, 8
NCORES = 8
NPC = N // NCORES  # batch elems per core
CH = 8             # H rows per chunk (PSUM bank = 448 fp32 cols)
NCHUNK = H // CH

F32 = mybir.dt.float32
BF16 = mybir.dt.bfloat16

TRACE = False
TRACE_DIR = None
LAST_EXEC_NS = None
LAST_RESULTS = None

_COMPILED = None


def _enable_trace_hook():
    """The agent image's ``antenv`` lacks ``axon_hooks``, so the boot-time
    NTFF hook registration silently degraded. Recreate the module and
    register the same ctypes-based hook; also skip the bucket upload."""
    import sys as _sys
    import types

    if "antenv.axon_hooks" not in _sys.modules:
        mod = types.ModuleType("antenv.axon_hooks")
        mod._hook = None

        def set_axon_ntff_profile_hook(h):
            mod._hook = h

        def get_axon_ntff_profile_hook():
            return mod._hook

        mod.set_axon_ntff_profile_hook = set_axon_ntff_profile_hook
        mod.get_axon_ntff_profile_hook = get_axon_ntff_profile_hook
        _sys.modules["antenv.axon_hooks"] = mod
        import antenv

        antenv.axon_hooks = mod

    from antenv.axon_hooks import get_axon_ntff_profile_hook as _get

    if _get() is None:
        from trn_agent_boot.trn_boot import _ntff_profile_via_ctypes

        hook = _ntff_profile_via_ctypes("/opt/axon/libaxon_pjrt.so")
        if hook is not None:
            _sys.modules["antenv.axon_hooks"].set_axon_ntff_profile_hook(hook)

    bass_utils.upload_artifacts = lambda tmpdir: f"local:{tmpdir}"


FOLD0 = 47         # fold region rows FOLD0..H-1; t1/DVE covers 0..FOLD0-1
FROWS = H - FOLD0  # 9 rows -> 504 fp32 cols, fits one PSUM bank


def _t3_fold_matmuls(pa, xc, wc_t, h0, rows):
    """Folded 7-tap conv for t3 rows h0..h0+rows-1 (row-clipped at the H
    borders) plus the t1-clip correction taps where the region touches
    h=0 or h=H-1.  Output row o reads x row h0+o+f-3."""
    mms = []
    # f=3 covers the full chunk -> emitted first (start=True)
    for f in (3, 0, 1, 2, 4, 5, 6):
        o_lo = max(0, 3 - f - h0)
        o_hi = min(rows, H + 3 - f - h0)
        if o_lo >= o_hi:
            continue
        r0 = h0 + o_lo + f - 3
        r1 = h0 + o_hi + f - 3
        mms.append((wc_t[:, f, :], xc[:, r0:r1, :], pa[:, o_lo:o_hi, :]))
    if h0 == 0:
        # fold wrongly includes t1[-1] at h=0
        for j in range(2):
            mms.append((wc_t[:, 7 + j, :], xc[:, j : j + 1, :], pa[:, 0:1, :]))
    if h0 + rows == H:
        # fold wrongly includes t1[56] at h=55
        for j in range(2):
            mms.append(
                (wc_t[:, 9 + j, :], xc[:, 54 + j : 55 + j, :], pa[:, rows - 1 : rows, :])
            )
    return mms


def _t1_matmuls(c, pa, xc, w5_t):
    """5-tap t1 conv for the 8-row chunk c (t1 rows 8c..8c+7), clipped at
    the top border.  Tap e=2 covers the full chunk and is emitted first."""
    h0 = c * CH
    mms = []
    for e in (2, 0, 1, 3, 4):
        o_lo = max(0, 2 - e - h0)
        o_hi = min(CH, H + 2 - e - h0)
        if o_lo >= o_hi:
            continue
        r0 = h0 + o_lo + e - 2
        r1 = h0 + o_hi + e - 2
        mms.append((w5_t[:, e, :], xc[:, r0:r1, :], pa[:, o_lo:o_hi, :]))
    return mms


def _t4_matmuls(c, pb, xc, w4_t):
    """t4 chunk: 3 width taps at offsets -2/0/+2, col-clipped at borders."""
    h0 = c * CH
    rows = xc[:, h0 : h0 + CH, :]
    return [
        (w4_t[:, 1, :], rows, pb[:]),                               # delta = 0
        (w4_t[:, 0, :], xc[:, h0 : h0 + CH, 0 : W - 2], pb[:, :, 2:W]),   # -2
        (w4_t[:, 2, :], xc[:, h0 : h0 + CH, 2:W], pb[:, :, 0 : W - 2]),   # +2
    ]


def _build():
    nc = bacc.Bacc(
        "TRN2",
        target_bir_lowering=False,
        debug=False,
        enable_asserts=False,
        num_devices=NCORES,
    )

    x_d = nc.dram_tensor("x_s", (NPC, C, H, W), BF16, kind="ExternalInput").ap()
    wc_d = nc.dram_tensor("wc", (C, 11, C), BF16, kind="ExternalInput").ap()
    w5_d = nc.dram_tensor("wc5", (C, 5, C), BF16, kind="ExternalInput").ap()
    w4_d = nc.dram_tensor("w4b", (C, 3, C), BF16, kind="ExternalInput").ap()
    w3_d = nc.dram_tensor("w3c", (C, 3), F32, kind="ExternalInput").ap()
    out_d = nc.dram_tensor("out", (NPC, C, H, W), BF16, kind="ExternalOutput").ap()

    NT1 = FOLD0 // CH + 1  # 6 t1 chunks covering rows 0..47
    # depthwise units (out-row range, ready-after-t1-chunk)
    units_mid = ((0, 24, 3), (24, FOLD0, 5))

    with tile.TileContext(nc) as tc:
        with (
            tc.tile_pool(name="wpool", bufs=1) as wpool,
            tc.tile_pool(name="xpool", bufs=1) as xpool,
            tc.tile_pool(name="t1pool", bufs=2) as t1pool,
            tc.tile_pool(name="t4pool", bufs=2) as t4pool,
            tc.tile_pool(name="tmpool", bufs=2) as tmpool,
            tc.tile_pool(name="opool", bufs=2) as opool,
            tc.tile_pool(name="psA", bufs=4, space="PSUM") as papool,
            tc.tile_pool(name="psB", bufs=3, space="PSUM") as pbpool,
            tc.tile_pool(name="psD", bufs=1, space="PSUM") as pdpool,
        ):
            # Dummy matmuls on a zeroed SBUF strip while the first DMAs
            # stream in: PE_HAM ungates the 2.4 GHz clock only after
            # ~3.4us of sustained activity.  The garbage results go to a
            # PSUM bank that is never read.  memset on GpSimd (an
            # early-ready engine the PE never waits for).
            dmy = wpool.tile([C, 448], BF16)
            nc.gpsimd.memset(dmy[:], 0.0)
            dps = pdpool.tile([C, 448], F32)
            for _ in range(8):
                nc.tensor.matmul(
                    dps[:], lhsT=dmy[:, 0:C], rhs=dmy[:], start=True, stop=True
                )

            wc_t = wpool.tile([C, 11, C], BF16)
            w5_t = wpool.tile([C, 5, C], BF16)
            w4_t = wpool.tile([C, 3, C], BF16)
            w3_t = wpool.tile([C, 3], F32)

            xcs = []
            for n in range(NPC):
                xc = xpool.tile([C, H, W], BF16, name=f"xc{n}")
                xcs.append(xc)
            # weights + first batch elem first (in pieces, so chunk-0
            # matmuls start early); the two loads gating the first real
            # matmul (wc5, x0a) go out in parallel on SyncE and ScalarE
            # (SyncE serializes issues at ~600ns each); later elems'
            # loads are issued from inside the chunk loop
            nc.sync.dma_start(w5_t[:], w5_d[:])
            nc.scalar.dma_start(xcs[0][:, 0:18, :], x_d[0, :, 0:18, :])
            nc.sync.dma_start(w4_t[:], w4_d[:])
            nc.sync.dma_start(w3_t[:], w3_d[:])
            nc.sync.dma_start(xcs[0][:, 18:34, :], x_d[0, :, 18:34, :])
            nc.sync.dma_start(xcs[0][:, 34:H, :], x_d[0, :, 34:H, :])
            nc.sync.dma_start(wc_t[:], wc_d[:])

            def emit_mms(mms):
                for i, (lhsT, rhs, outap) in enumerate(mms):
                    nc.tensor.matmul(
                        outap, lhsT=lhsT, rhs=rhs,
                        start=(i == 0), stop=(i == len(mms) - 1),
                    )

            # elements 0..NPC-2: hybrid -- plain t1 chunks for rows
            # 0..FOLD0-1 with the depthwise conv on DVE (mid-stream,
            # plenty of pipeline room), folded t3 for the last rows
            # (a third depthwise unit here overloads DVE/Pool and
            # stalls the PE through pool backpressure -- measured)
            for n in range(NPC - 1):
                xc = xcs[n]

                # t1 strip rows 0..48: strip row r holds t1 row r-1; row 0
                # is the depthwise conv's zero padding
                t1s = t1pool.tile([C, FOLD0 + 2, W], BF16, name="t1s")
                nc.gpsimd.memset(t1s[:, 0:1, :], 0.0)
                t4s = t4pool.tile([C, H, W], BF16, name="t4s")
                ot = opool.tile([C, H, W], BF16, name="ot")

                def macmul(unit):
                    # t3 rows r0..r1-1 = sum of three per-partition-scaled
                    # shifted t1 reads (tensor_scalar hits the DVE fast
                    # bf16 mode; scalar_tensor_tensor would not), then the
                    # combine against the t4 strip on the Pool engine
                    r0, r1, _ = unit
                    rr = r1 - r0
                    sa = tmpool.tile([C, 24, W], BF16, name="sa")
                    sb = tmpool.tile([C, 24, W], BF16, name="sb")
                    sc = tmpool.tile([C, 24, W], BF16, name="sc")
                    uu = tmpool.tile([C, 24, W], BF16, name="uu")
                    t3 = tmpool.tile([C, 24, W], BF16, name="t3")
                    nc.vector.tensor_scalar_mul(
                        sa[:, 0:rr, :], t1s[:, r0 : r0 + rr, :], w3_t[:, 0:1]
                    )
                    nc.vector.tensor_scalar_mul(
                        sb[:, 0:rr, :], t1s[:, r0 + 1 : r0 + 1 + rr, :], w3_t[:, 1:2]
                    )
                    nc.vector.tensor_scalar_mul(
                        sc[:, 0:rr, :], t1s[:, r0 + 2 : r0 + 2 + rr, :], w3_t[:, 2:3]
                    )
                    nc.vector.tensor_add(uu[:, 0:rr, :], sa[:, 0:rr, :], sb[:, 0:rr, :])
                    nc.vector.tensor_add(t3[:, 0:rr, :], uu[:, 0:rr, :], sc[:, 0:rr, :])
                    nc.gpsimd.tensor_mul(
                        ot[:, r0:r1, :], t3[:, 0:rr, :], t4s[:, r0:r1, :]
                    )

                for c in range(NT1):
                    # single shape/name so U-chunks and the fold chunks
                    # share one PSUM rotation (9 rows <= 1 bank)
                    pa = papool.tile([C, FROWS, W], F32, name="pa")
                    emit_mms(_t1_matmuls(c, pa, xc, w5_t))
                    pb = pbpool.tile([C, CH, W], F32)
                    emit_mms(_t4_matmuls(c, pb, xc, w4_t))
                    # later elements' x loads, issued mid-stream (before
                    # the out-DMA issues: SyncE executes in order and
                    # the out issues block on compute semaphores)
                    if (n, c) in ((0, 2), (0, 5), (1, 5)):
                        k = n + 1 if c == 2 else n + 2
                        nc.sync.dma_start(xcs[k][:], x_d[k])
                    # PSUM chunks -> bf16 SBUF strips on ScalarE
                    nc.scalar.copy(
                        t1s[:, c * CH + 1 : c * CH + 1 + CH, :], pa[:, 0:CH, :]
                    )
                    nc.scalar.copy(t4s[:, c * CH : (c + 1) * CH, :], pb[:])
                    for unit in units_mid:
                        if unit[2] == c:
                            macmul(unit)

                # fold region rows FOLD0..H-1: 7-tap folded t3 in one
                # PSUM chunk, multiplied directly against the t4 strip
                paf = papool.tile([C, FROWS, W], F32, name="pa")
                emit_mms(_t3_fold_matmuls(paf, xc, wc_t, FOLD0, FROWS))
                pb = pbpool.tile([C, CH, W], F32)
                emit_mms(_t4_matmuls(NCHUNK - 1, pb, xc, w4_t))
                nc.scalar.copy(t4s[:, (NCHUNK - 1) * CH : H, :], pb[:])
                nc.vector.tensor_mul(
                    ot[:, FOLD0:H, :], paf[:], t4s[:, FOLD0:H, :]
                )
                nc.sync.dma_start(out_d[n], ot[:])

            # last element: rows 0..22 as plain t1 with ONE depthwise
            # unit emitted early (c==2, ~6us before stream end -- unlike
            # late-emitted units this clears the DVE/Pool queues before
            # the tail); rows 23..55 folded.  The final fold chunk is
            # split [48:52]/[52:56] and interleaved with the last t4
            # matmuls so the t4 copy and first 4-row multiply overlap
            # the final PE work -- only one 4-row mul+DMA trails.
            n = NPC - 1
            xc = xcs[n]
            t1s = t1pool.tile([C, FOLD0 + 2, W], BF16, name="t1s")
            nc.gpsimd.memset(t1s[:, 0:1, :], 0.0)
            t4s = t4pool.tile([C, H, W], BF16, name="t4s")
            ot = opool.tile([C, H, W], BF16, name="ot")
            for c in range(3):  # t1 rows 0..23, t4 rows 0..24
                pa = papool.tile([C, FROWS, W], F32, name="pa")
                emit_mms(_t1_matmuls(c, pa, xc, w5_t))
                pb = pbpool.tile([C, CH, W], F32)
                emit_mms(_t4_matmuls(c, pb, xc, w4_t))
                nc.scalar.copy(
                    t1s[:, c * CH + 1 : c * CH + 1 + CH, :], pa[:, 0:CH, :]
                )
                nc.scalar.copy(t4s[:, c * CH : (c + 1) * CH, :], pb[:])
            # depthwise unit rows 0..22 (t3 row 22 reads t1 row 23)
            sa = tmpool.tile([C, 24, W], BF16, name="sa")
            sb = tmpool.tile([C, 24, W], BF16, name="sb")
            sc = tmpool.tile([C, 24, W], BF16, name="sc")
            uu = tmpool.tile([C, 24, W], BF16, name="uu")
            t3 = tmpool.tile([C, 24, W], BF16, name="t3")
            nc.vector.tensor_scalar_mul(sa[:, 0:23, :], t1s[:, 0:23, :], w3_t[:, 0:1])
            nc.vector.tensor_scalar_mul(sb[:, 0:23, :], t1s[:, 1:24, :], w3_t[:, 1:2])
            nc.vector.tensor_scalar_mul(sc[:, 0:23, :], t1s[:, 2:25, :], w3_t[:, 2:3])
            nc.vector.tensor_add(uu[:, 0:23, :], sa[:, 0:23, :], sb[:, 0:23, :])
            nc.vector.tensor_add(t3[:, 0:23, :], uu[:, 0:23, :], sc[:, 0:23, :])
            # mul on DVE, not Pool: the Pool queue (elems 0-2's unit
            # muls) runs right up to stream end, and this piece's DMA
            # would head-of-line block the remaining out issues on SyncE
            nc.vector.tensor_mul(ot[:, 0:23, :], t3[:, 0:23, :], t4s[:, 0:23, :])
            nc.sync.dma_start(out_d[n, :, 0:23, :], ot[:, 0:23, :])
            # fold chunks rows 23..47 paired with t4 chunks 3..5
            for h0 in (23, 31, 39):
                pa = papool.tile([C, FROWS, W], F32, name="pa")
                emit_mms(_t3_fold_matmuls(pa, xc, wc_t, h0, CH))
                tc4 = h0 // CH + 1
                pb = pbpool.tile([C, CH, W], F32)
                emit_mms(_t4_matmuls(tc4, pb, xc, w4_t))
                nc.scalar.copy(t4s[:, tc4 * CH : (tc4 + 1) * CH, :], pb[:])
                nc.vector.tensor_mul(
                    ot[:, h0 : h0 + CH, :], pa[:, 0:CH, :], t4s[:, h0 : h0 + CH, :]
                )
            nc.sync.dma_start(out_d[n, :, 23:47, :], ot[:, 23:47, :])
            # t4 chunk 6 first (so its ScalarE copy -- the gate for the
            # final multiplies -- runs during the last fold matmuls),
            # then rows 47:52, then rows 52:56 last
            pb = pbpool.tile([C, CH, W], F32)
            emit_mms(_t4_matmuls(NCHUNK - 1, pb, xc, w4_t))
            nc.scalar.copy(t4s[:, (NCHUNK - 1) * CH : H, :], pb[:])
            pa1 = papool.tile([C, FROWS, W], F32, name="pa")
            emit_mms(_t3_fold_matmuls(pa1, xc, wc_t, FOLD0, 5))
            nc.vector.tensor_mul(
                ot[:, FOLD0:52, :], pa1[:, 0:5, :], t4s[:, FOLD0:52, :]
            )
            nc.sync.dma_start(out_d[n, :, FOLD0:52, :], ot[:, FOLD0:52, :])
            pa2 = papool.tile([C, FROWS, W], F32, name="pa")
            emit_mms(_t3_fold_matmuls(pa2, xc, wc_t, 52, 4))
            nc.vector.tensor_mul(
                ot[:, 52:H, :], pa2[:, 0:4, :], t4s[:, 52:H, :]
            )
            nc.scalar.dma_start(out_d[n, :, 52:H, :], ot[:, 52:H, :])

    nc.compile()
    return nc


def _get_compiled():
    global _COMPILED
    if _COMPILED is None:
        _COMPILED = _build()
    return _COMPILED


def _prep_weights(w1, w3, w4):
    w1c = np.asarray(w1, dtype=np.float32)[:, :, :, 0]  # (co, ci, 5)
    w3c = np.asarray(w3, dtype=np.float32)[:, 0, :, 0]  # (co, 3)
    wc = np.zeros((C, 11, C), dtype=np.float32)         # (ci, tap, co)
    for d in range(3):
        for e in range(5):
            # wc[ci, d+e, co] += w1[co, ci, e] * w3[co, d]
            wc[:, d + e, :] += (w1c[:, :, e] * w3c[:, d][:, None]).T
    # border clip corrections (see _t3_fold_matmuls): taps 7,8 fix h=0;
    # taps 9,10 fix h=55
    for j, e in enumerate((3, 4)):
        wc[:, 7 + j, :] = -(w1c[:, :, e] * w3c[:, 0][:, None]).T
    for j, e in enumerate((0, 1)):
        wc[:, 9 + j, :] = -(w1c[:, :, e] * w3c[:, 2][:, None]).T
    wc5 = np.ascontiguousarray(w1c.transpose(1, 2, 0))  # (ci, tap, co)
    w4c = np.asarray(w4, dtype=np.float32)[:, :, 0, :]  # (ci, k, g)
    w4b = np.ascontiguousarray(np.tile(w4c, (1, 1, C // G)))  # (ci, k, 128)
    bf = ml_dtypes.bfloat16
    return (
        np.ascontiguousarray(wc).astype(bf),
        wc5.astype(bf),
        np.ascontiguousarray(w3c),
        w4b.astype(bf),
    )


def kernel(x, w1, w3, w4):
    global LAST_EXEC_NS, LAST_RESULTS
    nc = _get_compiled()
    xb = np.ascontiguousarray(np.asarray(x, dtype=np.float32)).astype(ml_dtypes.bfloat16)
    wc, wc5, w3c, w4b = _prep_weights(w1, w3, w4)

    in_maps = [
        {
            "x_s": np.ascontiguousarray(xb[i * NPC : (i + 1) * NPC]),
            "wc": wc,
            "wc5": wc5,
            "w3c": w3c,
            "w4b": w4b,
        }
        for i in range(NCORES)
    ]
    if TRACE:
        _enable_trace_hook()
    res = bass_utils.run_bass_kernel_spmd(
        nc,
        in_maps,
        core_ids=list(range(NCORES)),
        trace=TRACE,
        tmpdir=TRACE_DIR,
    )
    LAST_EXEC_NS = res.exec_time_ns
    LAST_RESULTS = res
    out = np.concatenate(
        [np.asarray(res.results[i]["out"]) for i in range(NCORES)], axis=0
    ).astype(np.float32)
    return out


# revision 48
# speedup vs baseline: 1.0124x; 1.0124x over previous
"""Trainium2 Bass kernel for dense_cnn problem.

Math (per batch element n, C=128 channels, H=W=56, G=8):
  t1 = conv_h(x, w1)          5-tap conv over H with full channel mixing
  t3 = dwconv_h(t1, w3)       3-tap depthwise conv over H
  t4[g] = sum_{c,k} x[c, h, w+2k-2] * w4[c,k,g]   (3 width taps, dil 2)
  out[c] = t3[c] * t4[c % 8]

Device strategy (data-parallel, 4 batch elems per core across 8 cores).
The PE matmul stream is the critical path (1 column/cycle @2.4GHz), so
the structure minimizes PE columns subject to the other engines keeping
up (their measured capacities are much lower than the docs' rooflines,
and they run several us behind their emission points near stream end):

  - Elements 0..2 (hybrid): rows 0..46 use plain 5-tap t1 matmuls; the
    3-tap depthwise conv runs on DVE as two ~24-row units (three
    per-partition-scaled tensor_scalar reads + two adds -- these hit
    the DVE fast bf16 mode; scalar_tensor_tensor does not) with the
    t3*t4 combine on the Pool engine.  These units complete mid-stream
    while the PE works on later elements.  Rows 47..55 use the folded
    7-tap t3 = (w3*w1) conv directly (one PSUM bank).  A third
    depthwise unit instead of the fold overloads DVE/Pool and stalls
    the PE (measured) -- this split balances them.
  - Element 3 (processed last) is FULLY folded: 7-tap t3 per 8-row
    chunk, ScalarE copies t4 to bf16, DVE multiplies.  Any depthwise
    unit emitted near stream end lands squarely in the tail; the fold
    keeps the tail to a copy+mul+DMA of the final rows, split 4+4 so
    only a 4-row transfer trails the last matmul.
  - t4: w4 broadcast to 128 output channels on the host -> 3 width-tap
    matmuls per chunk; the final combine is elementwise.
  - Matmuls in bf16; accumulation fp32 in PSUM.  x unpadded in SBUF;
    boundary taps are clipped matmuls.  Output DMA'd bf16 (half the
    bytes) and upconverted on the host.
  - x loads for elements 1-3 are issued mid-stream: issuing them up
    front steals HBM bandwidth from the pieces the PE needs first, and
    the resulting matmul stalls also reset the PE_HAM activity window,
    delaying the 1.2 -> 2.4 GHz clock boost.
  - 8 dummy 448-col matmuls (sized to end right as the first x piece
    and weights land, ~10.5us in) trip the PE_HAM clock gate while the
    first DMAs are still streaming in.
"""

import sys

sys.path.insert(0, "/opt/trn_rl_repo")

import ml_dtypes
import numpy as np

import concourse.bacc as bacc
import concourse.bass as bass
import concourse.mybir as mybir
import concourse.tile as tile
from concourse import bass_utils

N, C, H, W, G = 32, 128, 56, 56, 8
NCORES = 8
NPC = N // NCORES  # batch elems per core
CH = 8             # H rows per chunk (PSUM bank = 448 fp32 cols)
NCHUNK = H // CH

F32 = mybir.dt.float32
BF16 = mybir.dt.bfloat16

TRACE = False
TRACE_DIR = None
LAST_EXEC_NS = None
LAST_RESULTS = None

_COMPILED = None


def _enable_trace_hook():
    """The agent image's ``antenv`` lacks ``axon_hooks``, so the boot-time
    NTFF hook registration silently degraded. Recreate the module and
    register the same ctypes-based hook; also skip the bucket upload."""
    import sys as _sys
    import types

    if "antenv.axon_hooks" not in _sys.modules:
        mod = types.ModuleType("antenv.axon_hooks")
        mod._hook = None

        def set_axon_ntff_profile_hook(h):
            mod._hook = h

        def get_axon_ntff_profile_hook():
            return mod._hook

        mod.set_axon_ntff_profile_hook = set_axon_ntff_profile_hook
        mod.get_axon_ntff_profile_hook = get_axon_ntff_profile_hook
        _sys.modules["antenv.axon_hooks"] = mod
        import antenv

        antenv.axon_hooks = mod

    from antenv.axon_hooks import get_axon_ntff_profile_hook as _get

    if _get() is None:
        from trn_agent_boot.trn_boot import _ntff_profile_via_ctypes

        hook = _ntff_profile_via_ctypes("/opt/axon/libaxon_pjrt.so")
        if hook is not None:
            _sys.modules["antenv.axon_hooks"].set_axon_ntff_profile_hook(hook)

    bass_utils.upload_artifacts = lambda tmpdir: f"local:{tmpdir}"


FOLD0 = 47         # fold region rows FOLD0..H-1; t1/DVE covers 0..FOLD0-1
FROWS = H - FOLD0  # 9 rows -> 504 fp32 cols, fits one PSUM bank


def _t3_fold_matmuls(pa, xc, wc_t, h0, rows):
    """Folded 7-tap conv for t3 rows h0..h0+rows-1 (row-clipped at the H
    borders) plus the t1-clip correction taps where the region touches
    h=0 or h=H-1.  Output row o reads x row h0+o+f-3."""
    mms = []
    # f=3 covers the full chunk -> emitted first (start=True)
    for f in (3, 0, 1, 2, 4, 5, 6):
        o_lo = max(0, 3 - f - h0)
        o_hi = min(rows, H + 3 - f - h0)
        if o_lo >= o_hi:
            continue
        r0 = h0 + o_lo + f - 3
        r1 = h0 + o_hi + f - 3
        mms.append((wc_t[:, f, :], xc[:, r0:r1, :], pa[:, o_lo:o_hi, :]))
    if h0 == 0:
        # fold wrongly includes t1[-1] at h=0
        for j in range(2):
            mms.append((wc_t[:, 7 + j, :], xc[:, j : j + 1, :], pa[:, 0:1, :]))
    if h0 + rows == H:
        # fold wrongly includes t1[56] at h=55
        for j in range(2):
            mms.append(
                (wc_t[:, 9 + j, :], xc[:, 54 + j : 55 + j, :], pa[:, rows - 1 : rows, :])
            )
    return mms


def _t1_matmuls(c, pa, xc, w5_t):
    """5-tap t1 conv for the 8-row chunk c (t1 rows 8c..8c+7), clipped at
    the top border.  Tap e=2 covers the full chunk and is emitted first."""
    h0 = c * CH
    mms = []
    for e in (2, 0, 1, 3, 4):
        o_lo = max(0, 2 - e - h0)
        o_hi = min(CH, H + 2 - e - h0)
        if o_lo >= o_hi:
            continue
        r0 = h0 + o_lo + e - 2
        r1 = h0 + o_hi + e - 2
        mms.append((w5_t[:, e, :], xc[:, r0:r1, :], pa[:, o_lo:o_hi, :]))
    return mms


def _t4_matmuls(c, pb, xc, w4_t):
    """t4 chunk: 3 width taps at offsets -2/0/+2, col-clipped at borders."""
    h0 = c * CH
    rows = xc[:, h0 : h0 + CH, :]
    return [
        (w4_t[:, 1, :], rows, pb[:]),                               # delta = 0
        (w4_t[:, 0, :], xc[:, h0 : h0 + CH, 0 : W - 2], pb[:, :, 2:W]),   # -2
        (w4_t[:, 2, :], xc[:, h0 : h0 + CH, 2:W], pb[:, :, 0 : W - 2]),   # +2
    ]


def _build():
    nc = bacc.Bacc(
        "TRN2",
        target_bir_lowering=False,
        debug=False,
        enable_asserts=False,
        num_devices=NCORES,
    )

    x_d = nc.dram_tensor("x_s", (NPC, C, H, W), BF16, kind="ExternalInput").ap()
    wc_d = nc.dram_tensor("wc", (C, 11, C), BF16, kind="ExternalInput").ap()
    w5_d = nc.dram_tensor("wc5", (C, 5, C), BF16, kind="ExternalInput").ap()
    w4_d = nc.dram_tensor("w4b", (C, 3, C), BF16, kind="ExternalInput").ap()
    w3_d = nc.dram_tensor("w3c", (C, 3), F32, kind="ExternalInput").ap()
    out_d = nc.dram_tensor("out", (NPC, C, H, W), BF16, kind="ExternalOutput").ap()

    NT1 = FOLD0 // CH + 1  # 6 t1 chunks covering rows 0..47
    # depthwise units (out-row range, ready-after-t1-chunk)
    units_mid = ((0, 24, 3), (24, FOLD0, 5))

    with tile.TileContext(nc) as tc:
        with (
            tc.tile_pool(name="wpool", bufs=1) as wpool,
            tc.tile_pool(name="xpool", bufs=1) as xpool,
            tc.tile_pool(name="t1pool", bufs=2) as t1pool,
            tc.tile_pool(name="t4pool", bufs=2) as t4pool,
            tc.tile_pool(name="tmpool", bufs=2) as tmpool,
            tc.tile_pool(name="opool", bufs=2) as opool,
            tc.tile_pool(name="psA", bufs=4, space="PSUM") as papool,
            tc.tile_pool(name="psB", bufs=3, space="PSUM") as pbpool,
            tc.tile_pool(name="psD", bufs=1, space="PSUM") as pdpool,
        ):
            # Dummy matmuls on a zeroed SBUF strip while the first DMAs
            # stream in: PE_HAM ungates the 2.4 GHz clock only after
            # ~3.4us of sustained activity.  The garbage results go to a
            # PSUM bank that is never read.  memset on GpSimd (an
            # early-ready engine the PE never waits for).
            dmy = wpool.tile([C, 448], BF16)
            nc.gpsimd.memset(dmy[:], 0.0)
            dps = pdpool.tile([C, 448], F32)
            for _ in range(8):
                nc.tensor.matmul(
                    dps[:], lhsT=dmy[:, 0:C], rhs=dmy[:], start=True, stop=True
                )

            wc_t = wpool.tile([C, 11, C], BF16)
            w5_t = wpool.tile([C, 5, C], BF16)
            w4_t = wpool.tile([C, 3, C], BF16)
            w3_t = wpool.tile([C, 3], F32)

            xcs = []
            for n in range(NPC):
                xc = xpool.tile([C, H, W], BF16, name=f"xc{n}")
                xcs.append(xc)
            # weights + first batch elem first (in pieces, so chunk-0
            # matmuls start early); later elems' loads are issued from
            # inside the chunk loop.  (Issuing x0a from ScalarE in
            # parallel with SyncE was tried and measured slower.)
            nc.sync.dma_start(w5_t[:], w5_d[:])
            nc.sync.dma_start(xcs[0][:, 0:18, :], x_d[0, :, 0:18, :])
            nc.sync.dma_start(w4_t[:], w4_d[:])
            nc.sync.dma_start(w3_t[:], w3_d[:])
            nc.sync.dma_start(xcs[0][:, 18:34, :], x_d[0, :, 18:34, :])
            nc.sync.dma_start(xcs[0][:, 34:H, :], x_d[0, :, 34:H, :])
            nc.sync.dma_start(wc_t[:], wc_d[:])

            def emit_mms(mms):
                for i, (lhsT, rhs, outap) in enumerate(mms):
                    nc.tensor.matmul(
                        outap, lhsT=lhsT, rhs=rhs,
                        start=(i == 0), stop=(i == len(mms) - 1),
                    )

            # elements 0..NPC-2: hybrid -- plain t1 chunks for rows
            # 0..FOLD0-1 with the depthwise conv on DVE (mid-stream,
            # plenty of pipeline room), folded t3 for the last rows
            # (a third depthwise unit here overloads DVE/Pool and
            # stalls the PE through pool backpressure -- measured)
            for n in range(NPC - 1):
                xc = xcs[n]

                # t1 strip rows 0..48: strip row r holds t1 row r-1; row 0
                # is the depthwise conv's zero padding
                t1s = t1pool.tile([C, FOLD0 + 2, W], BF16, name="t1s")
                nc.gpsimd.memset(t1s[:, 0:1, :], 0.0)
                t4s = t4pool.tile([C, H, W], BF16, name="t4s")
                ot = opool.tile([C, H, W], BF16, name="ot")

                def macmul(unit):
                    # t3 rows r0..r1-1 = sum of three per-partition-scaled
                    # shifted t1 reads (tensor_scalar hits the DVE fast
                    # bf16 mode; scalar_tensor_tensor would not), then the
                    # combine against the t4 strip on the Pool engine
                    r0, r1, _ = unit
                    rr = r1 - r0
                    sa = tmpool.tile([C, 24, W], BF16, name="sa")
                    sb = tmpool.tile([C, 24, W], BF16, name="sb")
                    sc = tmpool.tile([C, 24, W], BF16, name="sc")
                    uu = tmpool.tile([C, 24, W], BF16, name="uu")
                    t3 = tmpool.tile([C, 24, W], BF16, name="t3")
                    nc.vector.tensor_scalar_mul(
                        sa[:, 0:rr, :], t1s[:, r0 : r0 + rr, :], w3_t[:, 0:1]
                    )
                    nc.vector.tensor_scalar_mul(
                        sb[:, 0:rr, :], t1s[:, r0 + 1 : r0 + 1 + rr, :], w3_t[:, 1:2]
                    )
                    nc.vector.tensor_scalar_mul(
                        sc[:, 0:rr, :], t1s[:, r0 + 2 : r0 + 2 + rr, :], w3_t[:, 2:3]
                    )
                    nc.vector.tensor_add(uu[:, 0:rr, :], sa[:, 0:rr, :], sb[:, 0:rr, :])
                    nc.vector.tensor_add(t3[:, 0:rr, :], uu[:, 0:rr, :], sc[:, 0:rr, :])
                    nc.gpsimd.tensor_mul(
                        ot[:, r0:r1, :], t3[:, 0:rr, :], t4s[:, r0:r1, :]
                    )

                for c in range(NT1):
                    # single shape/name so U-chunks and the fold chunks
                    # share one PSUM rotation (9 rows <= 1 bank)
                    pa = papool.tile([C, FROWS, W], F32, name="pa")
                    emit_mms(_t1_matmuls(c, pa, xc, w5_t))
                    pb = pbpool.tile([C, CH, W], F32)
                    emit_mms(_t4_matmuls(c, pb, xc, w4_t))
                    # later elements' x loads, issued mid-stream (before
                    # the out-DMA issues: SyncE executes in order and
                    # the out issues block on compute semaphores)
                    if (n, c) in ((0, 2), (0, 5), (1, 5)):
                        k = n + 1 if c == 2 else n + 2
                        nc.sync.dma_start(xcs[k][:], x_d[k])
                    # PSUM chunks -> bf16 SBUF strips on ScalarE
                    nc.scalar.copy(
                        t1s[:, c * CH + 1 : c * CH + 1 + CH, :], pa[:, 0:CH, :]
                    )
                    nc.scalar.copy(t4s[:, c * CH : (c + 1) * CH, :], pb[:])
                    for unit in units_mid:
                        if unit[2] == c:
                            macmul(unit)

                # fold region rows FOLD0..H-1: 7-tap folded t3 in one
                # PSUM chunk, multiplied directly against the t4 strip
                paf = papool.tile([C, FROWS, W], F32, name="pa")
                emit_mms(_t3_fold_matmuls(paf, xc, wc_t, FOLD0, FROWS))
                pb = pbpool.tile([C, CH, W], F32)
                emit_mms(_t4_matmuls(NCHUNK - 1, pb, xc, w4_t))
                nc.scalar.copy(t4s[:, (NCHUNK - 1) * CH : H, :], pb[:])
                nc.vector.tensor_mul(
                    ot[:, FOLD0:H, :], paf[:], t4s[:, FOLD0:H, :]
                )
                nc.sync.dma_start(out_d[n], ot[:])

            # last element: rows 0..22 as plain t1 with ONE depthwise
            # unit emitted early (c==2, ~6us before stream end -- unlike
            # late-emitted units this clears the DVE/Pool queues before
            # the tail); rows 23..55 folded.  The final fold chunk is
            # split [48:52]/[52:56] and interleaved with the last t4
            # matmuls so the t4 copy and first 4-row multiply overlap
            # the final PE work -- only one 4-row mul+DMA trails.
            n = NPC - 1
            xc = xcs[n]
            t1s = t1pool.tile([C, FOLD0 + 2, W], BF16, name="t1s")
            nc.gpsimd.memset(t1s[:, 0:1, :], 0.0)
            t4s = t4pool.tile([C, H, W], BF16, name="t4s")
            ot = opool.tile([C, H, W], BF16, name="ot")
            for c in range(3):  # t1 rows 0..23, t4 rows 0..24
                pa = papool.tile([C, FROWS, W], F32, name="pa")
                emit_mms(_t1_matmuls(c, pa, xc, w5_t))
                pb = pbpool.tile([C, CH, W], F32)
                emit_mms(_t4_matmuls(c, pb, xc, w4_t))
                nc.scalar.copy(
                    t1s[:, c * CH + 1 : c * CH + 1 + CH, :], pa[:, 0:CH, :]
                )
                nc.scalar.copy(t4s[:, c * CH : (c + 1) * CH, :], pb[:])
            # depthwise unit rows 0..22 (t3 row 22 reads t1 row 23)
            sa = tmpool.tile([C, 24, W], BF16, name="sa")
            sb = tmpool.tile([C, 24, W], BF16, name="sb")
            sc = tmpool.tile([C, 24, W], BF16, name="sc")
            uu = tmpool.tile([C, 24, W], BF16, name="uu")
            t3 = tmpool.tile([C, 24, W], BF16, name="t3")
            nc.vector.tensor_scalar_mul(sa[:, 0:23, :], t1s[:, 0:23, :], w3_t[:, 0:1])
            nc.vector.tensor_scalar_mul(sb[:, 0:23, :], t1s[:, 1:24, :], w3_t[:, 1:2])
            nc.vector.tensor_scalar_mul(sc[:, 0:23, :], t1s[:, 2:25, :], w3_t[:, 2:3])
            nc.vector.tensor_add(uu[:, 0:23, :], sa[:, 0:23, :], sb[:, 0:23, :])
            nc.vector.tensor_add(t3[:, 0:23, :], uu[:, 0:23, :], sc[:, 0:23, :])
            # mul on DVE, not Pool: the Pool queue (elems 0-2's unit
            # muls) runs right up to stream end, and this piece's DMA
            # would head-of-line block the remaining out issues on SyncE
            nc.vector.tensor_mul(ot[:, 0:23, :], t3[:, 0:23, :], t4s[:, 0:23, :])
            nc.sync.dma_start(out_d[n, :, 0:23, :], ot[:, 0:23, :])
            # fold chunks rows 23..47 paired with t4 chunks 3..5
            for h0 in (23, 31, 39):
                pa = papool.tile([C, FROWS, W], F32, name="pa")
                emit_mms(_t3_fold_matmuls(pa, xc, wc_t, h0, CH))
                tc4 = h0 // CH + 1
                pb = pbpool.tile([C, CH, W], F32)
                emit_mms(_t4_matmuls(tc4, pb, xc, w4_t))
                nc.scalar.copy(t4s[:, tc4 * CH : (tc4 + 1) * CH, :], pb[:])
                nc.vector.tensor_mul(
                    ot[:, h0 : h0 + CH, :], pa[:, 0:CH, :], t4s[:, h0 : h0 + CH, :]
                )
            nc.sync.dma_start(out_d[n, :, 23:47, :], ot[:, 23:47, :])
            # rows 47:52, then t4 chunk 6, then rows 52:56 last: the t4
            # copy and the first multiply overlap the final matmuls
            pa1 = papool.tile([C, FROWS, W], F32, name="pa")
            emit_mms(_t3_fold_matmuls(pa1, xc, wc_t, FOLD0, 5))
            pb = pbpool.tile([C, CH, W], F32)
            emit_mms(_t4_matmuls(NCHUNK - 1, pb, xc, w4_t))
            nc.scalar.copy(t4s[:, (NCHUNK - 1) * CH : H, :], pb[:])
            nc.vector.tensor_mul(
                ot[:, FOLD0:52, :], pa1[:, 0:5, :], t4s[:, FOLD0:52, :]
            )
            nc.sync.dma_start(out_d[n, :, FOLD0:52, :], ot[:, FOLD0:52, :])
            pa2 = papool.tile([C, FROWS, W], F32, name="pa")
            emit_mms(_t3_fold_matmuls(pa2, xc, wc_t, 52, 4))
            nc.vector.tensor_mul(
                ot[:, 52:H, :], pa2[:, 0:4, :], t4s[:, 52:H, :]
            )
            nc.scalar.dma_start(out_d[n, :, 52:H, :], ot[:, 52:H, :])

    nc.compile()
    return nc


def _get_compiled():
    global _COMPILED
    if _COMPILED is None:
        _COMPILED = _build()
    return _COMPILED


def _prep_weights(w1, w3, w4):
    w1c = np.asarray(w1, dtype=np.float32)[:, :, :, 0]  # (co, ci, 5)
    w3c = np.asarray(w3, dtype=np.float32)[:, 0, :, 0]  # (co, 3)
    wc = np.zeros((C, 11, C), dtype=np.float32)         # (ci, tap, co)
    for d in range(3):
        for e in range(5):
            # wc[ci, d+e, co] += w1[co, ci, e] * w3[co, d]
            wc[:, d + e, :] += (w1c[:, :, e] * w3c[:, d][:, None]).T
    # border clip corrections (see _t3_fold_matmuls): taps 7,8 fix h=0;
    # taps 9,10 fix h=55
    for j, e in enumerate((3, 4)):
        wc[:, 7 + j, :] = -(w1c[:, :, e] * w3c[:, 0][:, None]).T
    for j, e in enumerate((0, 1)):
        wc[:, 9 + j, :] = -(w1c[:, :, e] * w3c[:, 2][:, None]).T
    wc5 = np.ascontiguousarray(w1c.transpose(1, 2, 0))  # (ci, tap, co)
    w4c = np.asarray(w4, dtype=np.float32)[:, :, 0, :]  # (ci, k, g)
    w4b = np.ascontiguousarray(np.tile(w4c, (1, 1, C // G)))  # (ci, k, 128)
    bf = ml_dtypes.bfloat16
    return (
        np.ascontiguousarray(wc).astype(bf),
        wc5.astype(bf),
        np.ascontiguousarray(w3c),
        w4b.astype(bf),
    )


def kernel(x, w1, w3, w4):
    global LAST_EXEC_NS, LAST_RESULTS
    nc = _get_compiled()
    xb = np.ascontiguousarray(np.asarray(x, dtype=np.float32)).astype(ml_dtypes.bfloat16)
    wc, wc5, w3c, w4b = _prep_weights(w1, w3, w4)

    in_maps = [
        {
            "x_s": np.ascontiguousarray(xb[i * NPC : (i + 1) * NPC]),
            "wc": wc,
            "wc5": wc5,
            "w3c": w3c,
            "w4b": w4b,
        }
        for i in range(NCORES)
    ]
    if TRACE:
        _enable_trace_hook()
    res = bass_utils.run_bass_kernel_spmd(
        nc,
        in_maps,
        core_ids=list(range(NCORES)),
        trace=TRACE,
        tmpdir=TRACE_DIR,
    )
    LAST_EXEC_NS = res.exec_time_ns
    LAST_RESULTS = res
    out = np.concatenate(
        [np.asarray(res.results[i]["out"]) for i in range(NCORES)], axis=0
    ).astype(np.float32)
    return out
